# revision 1
# baseline (speedup 1.0000x reference)
"""Causal self-attention (B=4, T=2048, C=1024, H=16) on 8 TRN2 NeuronCores.

Sharding: core = (batch b, head-group g) with b in 0..3, g in 0..1.
Each core handles one batch element and 8 of the 16 heads (tensor-parallel
split of the QKV / proj weights).  The c_proj contraction is split over the
two head groups, so each core produces a partial [T, C] output; the host
sums the two partials per batch and adds b_proj (the "all-reduce" of the
TP sharding, done on the host during unsharding).

Device layout (per core) -- everything SBUF-resident, bf16 matmul inputs:
  xT    [C, T]        x[b]^T, host-transposed + bf16-cast
  wqkv  [C, 3*CL]     W_attn column slice for this head group (q scaled by
                      1/sqrt(D) on host), bf16
  qkvT = wqkv.T @ xT computed as [ch, t] tiles (q^T, k^T); v computed in
  natural [t, d] orientation as xT.T @ wv.
  Attention per head h: S^T[j, i] = k^T.T q^T  (contraction d=64, two heads
  packed on PE row-groups 0-63 / 64-127), structural causal masking (only
  lower-triangular j-tiles computed; diagonal tiles get an additive -1e9
  triangle constant), exp on ScalarE (no max subtraction -- scores are O(6)),
  P^T @ V' on PE with V' = [V | ones] so row 64 of the accumulator is the
  softmax denominator.  Normalization happens on the PSUM->SBUF copy.
  c_proj: out[t, :] += y^T.T @ Wp with K=128 channel tiles.
"""

import math

import ml_dtypes
import numpy as np

import concourse.bass as bass
import concourse.tile as tile
from concourse import bacc, mybir
from concourse.bass_utils import run_bass_kernel_spmd

# problem shape (hardcoded per the task contract)
B, T, C, H = 4, 2048, 1024, 16
D = C // H            # 64 head dim
NCORES = 8
HL = H // 2           # heads per core
CL = HL * D           # 512 local channels per core
NEG = -1.0e9

P = 128               # SBUF partitions
TI = 512              # query chunk (matmul moving dim)
TJ = 128              # key tile
CT = C // P           # 8 contraction tiles for the projections
NTT = T // P          # 16 t-tiles of 128
NIT = T // TI         # 4 query chunks
JQ = CL // P          # 4 channel tiles for q (and for k, and for y)
KC = CL // P          # 4 channel tiles in c_proj contraction
NOC = C // TI         # 2 output-column tiles in c_proj

FP32 = mybir.dt.float32
BF16 = mybir.dt.bfloat16
AF = mybir.ActivationFunctionType
ADD = mybir.AluOpType.add
MULT = mybir.AluOpType.mult


def _emit(tc, io):
    nc = tc.nc
    xT, wqkv, bqk, bv, wp, mtri, out = (
        io["xT"], io["wqkv"], io["bqk"], io["bv"], io["wp"], io["mtri"], io["out"]
    )

    with (
        tc.tile_pool(name="const", bufs=1) as cpool,
        tc.tile_pool(name="work", bufs=4) as wpool,
        tc.tile_pool(name="outp", bufs=3) as opool,
        tc.tile_pool(name="mm", bufs=2, space="PSUM") as mm_ps,
        tc.tile_pool(name="ps", bufs=2, space="PSUM") as s_ps,
        tc.tile_pool(name="po", bufs=2, space="PSUM") as o_ps,
    ):
        # persistent SBUF tensors
        xT_sb = cpool.tile([P, CT, T], BF16)
        wqkv_sb = cpool.tile([P, CT, 3 * CL], BF16)
        qT_sb = cpool.tile([P, JQ, T], BF16)
        kT_sb = cpool.tile([P, JQ, T], BF16)
        v_sb = cpool.tile([P, NTT, HL, D + 1], BF16)
        yT_sb = cpool.tile([P, JQ, T], BF16)
        wp_sb = cpool.tile([P, KC, C], BF16)
        mtri_sb = cpool.tile([P, P], FP32)
        bqk_sb = cpool.tile([P, 2 * JQ], FP32)
        bv_sb = cpool.tile([1, CL], FP32)
        bvb_sb = cpool.tile([P, CL], FP32)

        xT_d = xT.rearrange("(o p) t -> p o t", p=P)
        wqkv_d = wqkv.rearrange("(o p) j -> p o j", p=P)
        # DMA queue order = first-compute order: the small constants, x^T
        # chunk 0 + v/qk weights (prologue inputs), then the rest
        nc.sync.dma_start(bqk_sb[:], bqk[:])
        nc.sync.dma_start(bv_sb[:], bv[:])
        nc.sync.dma_start(mtri_sb[:], mtri[:])
        nc.sync.dma_start(xT_sb[:, :, 0:TI], xT_d[:, :, 0:TI])
        nc.sync.dma_start(wqkv_sb[:, :, 2 * CL :], wqkv_d[:, :, 2 * CL :])
        nc.sync.dma_start(wqkv_sb[:, :, : 2 * CL], wqkv_d[:, :, : 2 * CL])
        for tch in range(1, NIT):
            ts = slice(tch * TI, (tch + 1) * TI)
            nc.sync.dma_start(xT_sb[:, :, ts], xT_d[:, :, ts])
        nc.sync.dma_start(wp_sb[:], wp.rearrange("(o p) j -> p o j", p=P))
        nc.gpsimd.partition_broadcast(bvb_sb[:], bv_sb[:])

        # ones column of V' (softmax denominator accumulator)
        nc.vector.memset(v_sb[:, :, :, D : D + 1], 1.0)

        wv = wqkv_sb[:, :, 2 * CL : 3 * CL]

        def emit_v_tile(tt):
            """V in natural [t, d] orientation: V = xT.T @ wv, one t-tile."""
            pv = mm_ps.tile([P, CL], FP32, tag="mm")
            for o in range(CT):
                nc.tensor.matmul(
                    pv[:],
                    xT_sb[:, o, tt * P : (tt + 1) * P],
                    wv[:, o, :],
                    start=(o == 0),
                    stop=(o == CT - 1),
                )
            # copy + v-bias (broadcast along partitions beforehand)
            nc.vector.tensor_tensor(
                v_sb[:, tt, :, 0:D],
                pv.rearrange("p (h d) -> p h d", h=HL),
                bvb_sb.rearrange("p (h d) -> p h d", h=HL),
                ADD,
            )

        def emit_qkv_group(pr, g):
            """One [128-ch, 512-t] q^T or k^T tile for pair pr."""
            which, tc_ = divmod(g, NIT)
            jt = which * JQ + pr
            dst = qT_sb if which == 0 else kT_sb
            pq = mm_ps.tile([P, TI], FP32, tag="mm")
            for o in range(CT):
                nc.tensor.matmul(
                    pq[:],
                    wqkv_sb[:, o, jt * P : (jt + 1) * P],
                    xT_sb[:, o, tc_ * TI : (tc_ + 1) * TI],
                    start=(o == 0),
                    stop=(o == CT - 1),
                )
            nc.vector.tensor_scalar_add(
                dst[:, pr, tc_ * TI : (tc_ + 1) * TI], pq[:], bqk_sb[:, jt : jt + 1]
            )

        def emit_cproj_tile(pr, idx):
            """Pair pr's partial c_proj contribution for one [128-t, 512-c]
            output tile, accumulated into DRAM by the DMA engines."""
            tt, oc = divmod(idx, NOC)
            pc = mm_ps.tile([P, TI], FP32, tag="mm")
            nc.tensor.matmul(
                pc[:],
                yT_sb[:, pr, tt * P : (tt + 1) * P],
                wp_sb[:, pr, oc * TI : (oc + 1) * TI],
                start=True,
                stop=True,
            )
            ob = opool.tile([P, TI], FP32, tag="ob")
            nc.vector.tensor_copy(ob[:], pc[:])
            nc.sync.dma_start(
                out[pr, tt * P : (tt + 1) * P, oc * TI : (oc + 1) * TI], ob[:]
            )

        # c_proj tiles become available as pairs finish; they carry no
        # downstream dependencies, so they queue up and drain between
        # attention jt-steps to keep the PE busy (and HAM-warm) while
        # ScalarE works through the exp backlog.
        cproj_queue = []

        def drain_cproj(n):
            for _ in range(min(n, len(cproj_queue))):
                pr_, idx = cproj_queue.pop(0)
                emit_cproj_tile(pr_, idx)

        # pair 0 prologue: just enough for attention (0, it=0); the rest of
        # pair 0's q/k/V tiles interleave at it-chunk boundaries
        emit_qkv_group(0, 0)        # q cols [0:512]
        emit_qkv_group(0, NIT)      # k cols [0:512]
        for tt in range(4):
            emit_v_tile(tt)

        for pr in range(JQ):  # 4 head pairs; pair pr = local heads 2pr, 2pr+1
            # ---- attention for the head pair (pair pr+1's q^T/k^T matmuls
            # are interleaved per it-chunk to fill PE gaps while ScalarE
            # works through the exp backlog) ----
            for it in range(NIT):
                po_e = o_ps.tile([P, TI], FP32, tag="po")
                po_o = o_ps.tile([P, TI], FP32, tag="po")
                njt = (it + 1) * (TI // TJ)
                for jt in range(njt):
                    delta = jt * TJ - it * TI
                    lo = max(delta, 0)
                    # merged even/odd score tile: even head in cols 0:TI
                    # (PSUM bank 0), odd head in cols TI:2*TI (bank 1)
                    ps2 = s_ps.tile([P, 2 * TI], FP32, tag="ps")
                    # S^T = k^T.T @ q^T, contraction d=64; the two heads of
                    # the pair sit on PE row groups 0-63 / 64-127 and run
                    # concurrently.
                    nc.tensor.matmul(
                        ps2[:, lo:TI],
                        kT_sb[0:D, pr, jt * TJ : (jt + 1) * TJ],
                        qT_sb[0:D, pr, it * TI + lo : (it + 1) * TI],
                        start=True,
                        stop=True,
                    )
                    nc.tensor.matmul(
                        ps2[:, TI + lo : 2 * TI],
                        kT_sb[D:P, pr, jt * TJ : (jt + 1) * TJ],
                        qT_sb[D:P, pr, it * TI + lo : (it + 1) * TI],
                        start=True,
                        stop=True,
                        tile_position=(D, 0),
                    )
                    ps2v = ps2.rearrange("p (b c) -> p b c", b=2)
                    if delta >= 0:  # diagonal tile: strict upper triangle -> -1e9
                        nc.vector.tensor_tensor(
                            ps2v[:, :, delta : delta + TJ],
                            ps2v[:, :, delta : delta + TJ],
                            mtri_sb[:, None, :].to_broadcast((P, 2, TJ)),
                            ADD,
                        )
                    p2 = wpool.tile([P, 2 * TI], BF16, tag="p")
                    p2v = p2.rearrange("p (b c) -> p b c", b=2)
                    # columns [0:lo) are fully masked and the PV matmuls
                    # only read [lo:], so exp is restricted and no memset
                    # is needed
                    if lo > 0:
                        nc.scalar.activation(
                            p2v[:, :, lo:TI], ps2v[:, :, lo:TI], AF.Exp
                        )
                    else:
                        nc.scalar.activation(p2[:], ps2[:], AF.Exp)
                    first, last = (jt == 0), (jt == njt - 1)
                    nc.tensor.matmul(
                        po_e[0 : D + 1, lo:TI],
                        v_sb[:, jt, 2 * pr, :],
                        p2[:, lo:TI],
                        start=first,
                        stop=last,
                    )
                    nc.tensor.matmul(
                        po_o[0 : D + 1, lo:TI],
                        v_sb[:, jt, 2 * pr + 1, :],
                        p2[:, TI + lo : 2 * TI],
                        start=first,
                        stop=last,
                    )
                # Drain the PV accumulators to SBUF right away (frees the
                # PSUM banks for the next it-chunk), then normalize from the
                # SBUF copy: row D is the softmax denominator; 1/x computed
                # on ScalarE as Square(Abs_reciprocal_sqrt(x)) (DVE's
                # iterative-divide RECIPROCAL is ~6.4 cyc/elem and was the
                # top Vector-engine cost).  partition_broadcast's gpsimd
                # ucode reads the source with Q7 core 0, so the reciprocal
                # row is DMA'd to partition 0 first.
                islice = slice(it * TI, (it + 1) * TI)
                for po_x, parity in ((po_e, 0), (po_o, 1)):
                    osb = wpool.tile([P, TI], FP32, tag="osb")
                    nc.vector.tensor_copy(osb[0 : D + 1, :], po_x[0 : D + 1, :])
                    rec = wpool.tile([P, TI], FP32, tag="rec")
                    rb = wpool.tile([P, TI], FP32, tag="rb")
                    # 1/x as exp(-ln(x)): Ln and Exp live in the same ACT
                    # table set, so this costs no ACT_TABLE_LOAD switches
                    # against the softmax exps (unlike Rsqrt/Reciprocal).
                    nc.scalar.activation(rb[D : D + 1, :], osb[D : D + 1, :], AF.Ln)
                    nc.scalar.activation(
                        rec[D : D + 1, :], rb[D : D + 1, :], AF.Exp, scale=-1.0
                    )
                    nc.sync.dma_start(rec[0:1, :], rec[D : D + 1, :])
                    nc.gpsimd.partition_broadcast(rb[0:D, :], rec[0:1, :])
                    if parity == 0:
                        nc.vector.tensor_tensor(
                            yT_sb[0:D, pr, islice], osb[0:D, :], rb[0:D, :], MULT
                        )
                    else:
                        tmp = wpool.tile([D, TI], BF16, tag="tmp")
                        nc.vector.tensor_tensor(tmp[:], osb[0:D, :], rb[0:D, :], MULT)
                        # odd head's y^T lives on partitions 64-127:
                        # cross-partition move must go through DMA
                        nc.sync.dma_start(yT_sb[D:P, pr, islice], tmp[:])
                if pr == 0 and it + 1 < NIT:
                    # rest of pair 0's own q/k/V tiles, just in time
                    emit_qkv_group(0, it + 1)
                    emit_qkv_group(0, NIT + it + 1)
                    for tt in range(4 * (it + 1), 4 * (it + 2)):
                        emit_v_tile(tt)
                if pr + 1 < JQ:
                    emit_qkv_group(pr + 1, 2 * it)
                    emit_qkv_group(pr + 1, 2 * it + 1)
                # this it-chunk's y^T rows are final: queue their c_proj tiles
                cproj_queue.extend(
                    (pr, tt * NOC + oc)
                    for tt in range(4 * it, 4 * (it + 1))
                    for oc in range(NOC)
                )
                drain_cproj(NTT * NOC // NIT)

        drain_cproj(len(cproj_queue))


def build_nc():
    nc = bacc.Bacc("TRN2", target_bir_lowering=False, debug=False)
    io = {
        "xT": nc.dram_tensor("xT", [C, T], BF16, kind="ExternalInput").ap(),
        "wqkv": nc.dram_tensor("wqkv", [C, 3 * CL], BF16, kind="ExternalInput").ap(),
        "bqk": nc.dram_tensor("bqk", [P, 2 * JQ], FP32, kind="ExternalInput").ap(),
        "bv": nc.dram_tensor("bv", [1, CL], FP32, kind="ExternalInput").ap(),
        "wp": nc.dram_tensor("wp", [CL, C], BF16, kind="ExternalInput").ap(),
        "mtri": nc.dram_tensor("mtri", [P, P], FP32, kind="ExternalInput").ap(),
        # one partial [T, C] per head pair; the host sums them (cheaper
        # than DMA-accumulate, which runs far below line rate)
        "out": nc.dram_tensor("out", [JQ, T, C], FP32, kind="ExternalOutput").ap(),
    }
    with tile.TileContext(nc) as tc:
        _emit(tc, io)
    # The act-table-load pass assigns each activation the FIRST table set
    # containing its function, so Exp->'exp_and_others' and
    # Ln->'natural_log' alternate (a 1.3us ACT_TABLE_LOAD per switch, ~50
    # switches).  Restrict the choice to 'natural_log_exp_and_others'
    # (which holds every function this kernel uses) so exactly one table
    # load is emitted.  Set ids stay aligned with act_info.json because
    # the dict keeps all entries in order.
    orig_tables = bacc.get_activation_tables

    def _combined_only(arch):
        t = orig_tables(arch)
        return {
            name: (funcs if name == "natural_log_exp_and_others" else set())
            for name, funcs in t.items()
        }

    bacc.get_activation_tables = _combined_only
    try:
        nc.compile()
    finally:
        bacc.get_activation_tables = orig_tables
    return nc


def make_in_maps(x, W_attn, b_attn, W_proj):
    """Per-core input dicts: core = 2*batch + head_group."""
    bf = ml_dtypes.bfloat16
    scale = np.float32(1.0 / math.sqrt(D))
    mtri = np.where(
        np.arange(P)[None, :] < np.arange(P)[:, None],
        np.float32(NEG),
        np.float32(0.0),
    ).astype(np.float32)
    in_maps = []
    for core in range(NCORES):
        b, g = divmod(core, 2)
        hs = slice(g * CL, (g + 1) * CL)
        wq = (W_attn[:, 0:C][:, hs] * scale).astype(bf)
        wk = W_attn[:, C : 2 * C][:, hs].astype(bf)
        wv = W_attn[:, 2 * C : 3 * C][:, hs].astype(bf)
        bq = (b_attn[0:C][hs] * scale).astype(np.float32)
        bk = b_attn[C : 2 * C][hs].astype(np.float32)
        bv = b_attn[2 * C : 3 * C][hs].astype(np.float32)
        in_maps.append(
            {
                "xT": np.ascontiguousarray(x[b].T).astype(bf),
                "wqkv": np.ascontiguousarray(np.concatenate([wq, wk, wv], axis=1)),
                "bqk": np.ascontiguousarray(
                    np.concatenate([bq, bk]).reshape(2 * JQ, P).T
                ),
                "bv": bv.reshape(1, CL),
                "wp": np.ascontiguousarray(W_proj[hs, :]).astype(bf),
                "mtri": mtri,
            }
        )
    return in_maps


def combine_outputs(results, b_proj):
    out = np.empty((B, T, C), np.float32)
    for b in range(B):
        acc = results[2 * b]["out"].sum(axis=0, dtype=np.float32)
        acc += results[2 * b + 1]["out"].sum(axis=0, dtype=np.float32)
        acc += b_proj.astype(np.float32)[None, :]
        out[b] = acc
    return out


def _mask_is_causal(mask):
    if mask.shape != (B, T, T):
        return False
    tril = np.tril(np.ones((T, T), np.float32))
    return all(np.array_equal(np.asarray(mask[b]), tril) for b in range(B))


def _numpy_fallback(x, mask, W_attn, b_attn, W_proj, b_proj):
    # generic-mask fallback (never hit for the causal reference inputs)
    out = np.empty((B, T, C), np.float32)
    for b in range(B):
        qkv = x[b] @ W_attn + b_attn
        q, k, v = np.split(qkv, 3, axis=-1)
        q = q.reshape(T, H, D)
        k = k.reshape(T, H, D)
        v = v.reshape(T, H, D)
        y = np.empty((T, H, D), np.float32)
        for h in range(H):
            s = (q[:, h] @ k[:, h].T) / math.sqrt(D)
            s = s + NEG * (1.0 - mask[b])
            s = s - s.max(-1, keepdims=True)
            p = np.exp(s)
            p /= p.sum(-1, keepdims=True)
            y[:, h] = p @ v[:, h]
        out[b] = y.reshape(T, C) @ W_proj + b_proj
    return out


_NC = None


def kernel(x, mask, W_attn, b_attn, W_proj, b_proj):
    global _NC
    x = np.asarray(x, dtype=np.float32)
    mask = np.asarray(mask)
    W_attn = np.asarray(W_attn, dtype=np.float32)
    b_attn = np.asarray(b_attn, dtype=np.float32)
    W_proj = np.asarray(W_proj, dtype=np.float32)
    b_proj = np.asarray(b_proj, dtype=np.float32)

    if not _mask_is_causal(mask):
        return _numpy_fallback(x, mask, W_attn, b_attn, W_proj, b_proj)

    if _NC is None:
        _NC = build_nc()
    in_maps = make_in_maps(x, W_attn, b_attn, W_proj)
    res = run_bass_kernel_spmd(_NC, in_maps, core_ids=list(range(NCORES)))
    return combine_outputs(res.results, b_proj)



# revision 4
# speedup vs baseline: 1.2115x; 1.2115x over previous
"""Causal self-attention (B=4, T=2048, C=1024, H=16) on 8 TRN2 NeuronCores.

Sharding: core = (batch b, head-group g) with b in 0..3, g in 0..1.
Each core handles one batch element and 8 of the 16 heads (tensor-parallel
split of the QKV / proj weights).  Each core produces one partial [T, C]
output (c_proj contracted over its 8 heads, accumulated in PSUM across the
4 head pairs); the host sums the two per-batch partials and adds b_proj.

Device layout (per core) -- everything SBUF-resident, bf16 matmul inputs:
  xT    [C, T]        x[b]^T, host-transposed + bf16-cast
  wqkv  [C, 3*CL]     W_attn column slice for this head group (q scaled by
                      1/sqrt(D) on host), bf16
  qkvT = wqkv.T @ xT computed as [ch, t] tiles (q^T, k^T); v computed in
  natural [t, d] orientation as xT.T @ wv.
  Attention per head pair: S^T[j, i] = k^T.T q^T (contraction d=64, two
  heads packed on PE row-groups 0-63 / 64-127 running concurrently),
  structural causal masking (only lower-triangular j-tiles computed;
  diagonal tiles get an additive -1e9 triangle constant), exp on ScalarE
  (no max subtraction -- scores are O(6)), P^T @ V' on PE with
  V' = [V | ones] so row 64 of the accumulator is the softmax denominator.

  Schedule is query-chunk-outer (it = chunk of 512 queries): for each it,
  the 4 head pairs run their attention for that chunk; c_proj for chunk
  it-1 (contraction over all 4 pairs, K=512 accumulated in PSUM) drains as
  PE filler between attention slots, together with the next chunk's q/k
  projections and V tiles.  This keeps ScalarE (the exp stream, ~1.15us
  per slot vs ~0.64us of PE work) saturated while the PE runs projection
  matmuls in the gaps, and keeps the PE HAM-warm (no >3us idle windows).

  Normalization: ln(den) read directly from the PSUM accumulator row 64,
  rec = exp(-ln(den)) (Ln and Exp share one ACT table set), DMA of the rec
  row to partition 0, gpsimd partition-broadcast to 64 partitions, then
  one DVE multiply per head; the unnormalized y rows are copied to SBUF
  right after the last PV matmul so the single PSUM accumulator can be
  reused by the next head pair immediately.
"""

import math

import ml_dtypes
import numpy as np

import concourse.bass as bass
import concourse.tile as tile
from concourse import bacc, mybir
from concourse.bass_utils import run_bass_kernel_spmd

# problem shape (hardcoded per the task contract)
B, T, C, H = 4, 2048, 1024, 16
D = C // H            # 64 head dim
NCORES = 8
HL = H // 2           # heads per core
CL = HL * D           # 512 local channels per core
NEG = -1.0e9

P = 128               # SBUF partitions
TI = 512              # query chunk (matmul moving dim)
TJ = 128              # key tile
CT = C // P           # 8 contraction tiles for the projections
NTT = T // P          # 16 t-tiles of 128
NIT = T // TI         # 4 query chunks
JQ = CL // P          # 4 channel tiles for q (and for k, and for y)
KC = CL // P          # 4 channel tiles in c_proj contraction
NOC = C // TI         # 2 output-column tiles in c_proj

FP32 = mybir.dt.float32
BF16 = mybir.dt.bfloat16
AF = mybir.ActivationFunctionType
ADD = mybir.AluOpType.add
MULT = mybir.AluOpType.mult


def _emit(tc, io):
    nc = tc.nc
    xT, wqkv, bqk, bv, wp, mtri, out = (
        io["xT"], io["wqkv"], io["bqk"], io["bv"], io["wp"], io["mtri"], io["out"]
    )

    with (
        tc.tile_pool(name="const", bufs=1) as cpool,
        tc.tile_pool(name="work", bufs=4) as wpool,
        tc.tile_pool(name="epi", bufs=2) as epool,
        tc.tile_pool(name="outp", bufs=3) as opool,
        tc.tile_pool(name="ps", bufs=2, space="PSUM") as s_ps,
        tc.tile_pool(name="po", bufs=1, space="PSUM") as o_ps,
        tc.tile_pool(name="mm", bufs=1, space="PSUM") as mm_ps,
        tc.tile_pool(name="cp", bufs=1, space="PSUM") as cp_ps,
    ):
        # persistent SBUF tensors
        xT_sb = cpool.tile([P, CT, T], BF16)
        wqkv_sb = cpool.tile([P, CT, 3 * CL], BF16)
        qT_sb = cpool.tile([P, JQ, T], BF16)
        kT_sb = cpool.tile([P, JQ, T], BF16)
        v_sb = cpool.tile([P, NTT, HL, D + 1], BF16)
        yT_sb = cpool.tile([P, JQ, T], BF16)
        wp_sb = cpool.tile([P, KC, C], BF16)
        mtri_sb = cpool.tile([P, P], FP32)
        bqk_sb = cpool.tile([P, 2 * JQ], FP32)
        bv_sb = cpool.tile([1, CL], FP32)
        bvb_sb = cpool.tile([P, CL], FP32)

        xT_d = xT.rearrange("(o p) t -> p o t", p=P)
        wqkv_d = wqkv.rearrange("(o p) j -> p o j", p=P)

        # prologue DMAs in first-compute order: tiny constants, then the
        # pair-0 q/k weight column slices + x^T chunk 0 (first matmuls),
        # v weights, then the rest interleaved so each pair / chunk lands
        # just before its first use.  Total input is ~8.4MB = ~23us at HBM
        # rate; fine-grained ordering lets compute start at ~4us.
        nc.sync.dma_start(bqk_sb[:], bqk[:])
        nc.sync.dma_start(bv_sb[:], bv[:])
        nc.sync.dma_start(mtri_sb[:], mtri[:])

        def dma_wslice(which, pr):
            j0 = which * CL + pr * P
            nc.sync.dma_start(
                wqkv_sb[:, :, j0 : j0 + P], wqkv_d[:, :, j0 : j0 + P]
            )

        nc.sync.dma_start(xT_sb[:, :, 0:TI], xT_d[:, :, 0:TI])
        dma_wslice(1, 0)  # w_k pair 0
        dma_wslice(0, 0)  # w_q pair 0
        nc.sync.dma_start(wqkv_sb[:, :, 2 * CL :], wqkv_d[:, :, 2 * CL :])  # w_v
        dma_wslice(1, 1)
        dma_wslice(0, 1)
        nc.sync.dma_start(xT_sb[:, :, TI : 2 * TI], xT_d[:, :, TI : 2 * TI])
        dma_wslice(1, 2)
        dma_wslice(0, 2)
        dma_wslice(1, 3)
        dma_wslice(0, 3)
        nc.sync.dma_start(xT_sb[:, :, 2 * TI : 3 * TI], xT_d[:, :, 2 * TI : 3 * TI])
        nc.sync.dma_start(wp_sb[:], wp.rearrange("(o p) j -> p o j", p=P))
        nc.sync.dma_start(xT_sb[:, :, 3 * TI :], xT_d[:, :, 3 * TI :])

        nc.gpsimd.partition_broadcast(bvb_sb[:], bv_sb[:])
        # ones column of V' (softmax denominator accumulator)
        nc.vector.memset(v_sb[:, :, :, D : D + 1], 1.0)

        wv = wqkv_sb[:, :, 2 * CL : 3 * CL]

        # ---- PE filler: projection / c_proj work queued as ~1-matmul
        # items and drained between attention slots.  Items carry a key on
        # their last (finalizing) op so attention slots can force-drain
        # their producers before being emitted -- the Tile framework only
        # tracks dependencies in emission order, so a consumer emitted
        # before its producer would silently read stale SBUF. ----
        queue = []
        done = set()
        slots_left = [sum(4 * (it + 1) for it in range(NIT)) * JQ + 4 * NIT]

        def drain(n):
            for _ in range(min(n, len(queue))):
                key, f = queue.pop(0)
                f()
                if key is not None:
                    done.add(key)

        def drain_until(key):
            while key not in done:
                assert queue, f"filler item {key} was never enqueued"
                k, f = queue.pop(0)
                f()
                if k is not None:
                    done.add(k)

        def v_tile_items(tt):
            """V in natural [t, d] orientation: V = xT.T @ wv, one t-tile."""
            state = {}

            def mk(o):
                def f():
                    if o == 0:
                        state["t"] = mm_ps.tile([P, CL], FP32, tag="mm", name="vmm")
                    nc.tensor.matmul(
                        state["t"][:],
                        xT_sb[:, o, tt * P : (tt + 1) * P],
                        wv[:, o, :],
                        start=(o == 0),
                        stop=(o == CT - 1),
                    )
                return f

            items = [(None, mk(o)) for o in range(CT)]

            def bias():
                nc.vector.tensor_tensor(
                    v_sb[:, tt, :, 0:D],
                    state["t"].rearrange("p (h d) -> p h d", h=HL),
                    bvb_sb.rearrange("p (h d) -> p h d", h=HL),
                    ADD,
                )

            items.append((("v", tt), bias))
            return items

        def qkv_group_items(pr, which, tch):
            """One [128-ch, 512-t] q^T (which=0) or k^T (which=1) tile."""
            jt = which * JQ + pr
            dst = qT_sb if which == 0 else kT_sb
            state = {}

            def mk(o):
                def f():
                    if o == 0:
                        state["t"] = mm_ps.tile([P, TI], FP32, tag="mm", name="qkmm")
                    nc.tensor.matmul(
                        state["t"][:],
                        wqkv_sb[:, o, jt * P : (jt + 1) * P],
                        xT_sb[:, o, tch * TI : (tch + 1) * TI],
                        start=(o == 0),
                        stop=(o == CT - 1),
                    )
                return f

            items = [(None, mk(o)) for o in range(CT)]

            def bias():
                nc.vector.tensor_scalar_add(
                    dst[:, pr, tch * TI : (tch + 1) * TI],
                    state["t"][:],
                    bqk_sb[:, jt : jt + 1],
                )

            items.append((("qkv", which, pr, tch), bias))
            return items

        def cproj_items(tt, oc):
            """One [128-t, 512-c] c_proj output tile, K=512 accumulated in
            PSUM over the 4 head pairs, stored to DRAM as bf16."""
            state = {}

            def mk(pr):
                def f():
                    if pr == 0:
                        state["t"] = cp_ps.tile([P, TI], FP32, tag="cp", name="cpmm")
                    nc.tensor.matmul(
                        state["t"][:],
                        yT_sb[:, pr, tt * P : (tt + 1) * P],
                        wp_sb[:, pr, oc * TI : (oc + 1) * TI],
                        start=(pr == 0),
                        stop=(pr == JQ - 1),
                    )
                return f

            items = [(None, mk(pr)) for pr in range(JQ)]

            def store():
                ob = opool.tile([P, TI], BF16, tag="ob", name="ob")
                nc.vector.tensor_copy(ob[:], state["t"][:])
                nc.sync.dma_start(
                    out[tt * P : (tt + 1) * P, oc * TI : (oc + 1) * TI], ob[:]
                )

            items.append((None, store))
            return items

        # ---- attention slot + unit epilogue ----
        def slot(pr, it, jt, njt, po):
            # force-emit this slot's producers (emission order = the only
            # dependency order Tile sees)
            drain_until(("qkv", 0, pr, it))
            drain_until(("qkv", 1, pr, jt * TJ // TI))
            drain_until(("v", jt))
            delta = jt * TJ - it * TI
            lo = max(delta, 0)
            ps = s_ps.tile([P, 2, TI], FP32, tag="ps")
            # S^T = k^T.T @ q^T, contraction d=64; the two heads of the
            # pair sit on PE row groups 0-63 / 64-127 and run concurrently.
            nc.tensor.matmul(
                ps[:, 0, lo:TI],
                kT_sb[0:D, pr, jt * TJ : (jt + 1) * TJ],
                qT_sb[0:D, pr, it * TI + lo : (it + 1) * TI],
                start=True,
                stop=True,
            )
            nc.tensor.matmul(
                ps[:, 1, lo:TI],
                kT_sb[D:P, pr, jt * TJ : (jt + 1) * TJ],
                qT_sb[D:P, pr, it * TI + lo : (it + 1) * TI],
                start=True,
                stop=True,
                tile_position=(D, 0),
            )
            if delta >= 0:  # diagonal tile: strict upper triangle -> -1e9
                nc.vector.tensor_tensor(
                    ps[:, :, delta : delta + TJ],
                    ps[:, :, delta : delta + TJ],
                    mtri_sb[:, None, :].to_broadcast((P, 2, TJ)),
                    ADD,
                )
            p2 = wpool.tile([P, 2, TI], BF16, tag="p2")
            # columns [0:lo) are fully masked and the PV matmuls only read
            # [lo:], so exp is restricted and no memset is needed
            if lo > 0:
                nc.scalar.activation(p2[:, :, lo:TI], ps[:, :, lo:TI], AF.Exp)
            else:
                nc.scalar.activation(p2[:], ps[:], AF.Exp)
            first, last = (jt == 0), (jt == njt - 1)
            nc.tensor.matmul(
                po[0 : D + 1, 0, lo:TI],
                v_sb[:, jt, 2 * pr, :],
                p2[:, 0, lo:TI],
                start=first,
                stop=last,
            )
            nc.tensor.matmul(
                po[0 : D + 1, 1, lo:TI],
                v_sb[:, jt, 2 * pr + 1, :],
                p2[:, 1, lo:TI],
                start=first,
                stop=last,
            )

        def epilogue(pr, it, po):
            """Normalize the pair's y^T rows for this it-chunk."""
            islice = slice(it * TI, (it + 1) * TI)
            # free the PSUM accumulator fast: plain copy of y-hat + den
            osb = epool.tile([D + 1, 2, TI], FP32, tag="osb")
            nc.vector.tensor_copy(osb[:], po[0 : D + 1, :, :])
            # rec = exp(-ln(den)) -- Ln/Exp share one ACT table set, and
            # Ln reads the denominator row straight from PSUM (both
            # parities in one [1, 1024] pass: the two po banks are
            # adjacent inside the single [128, 2, 512] accumulator tile).
            rl = epool.tile([D + 1, 2, TI], FP32, tag="rl")
            nc.scalar.activation(rl[D : D + 1, :, :], po[D : D + 1, :, :], AF.Ln)
            rc = epool.tile([D + 1, 2, TI], FP32, tag="rc")
            nc.scalar.activation(
                rc[D : D + 1, :, :], rl[D : D + 1, :, :], AF.Exp, scale=-1.0
            )
            # partition_broadcast's gpsimd ucode reads the source with Q7
            # core 0, so the reciprocal row is DMA'd to partition 0 first.
            nc.sync.dma_start(rc[0:1, :, :], rc[D : D + 1, :, :])
            rbb = epool.tile([D, 2, TI], FP32, tag="rbb")
            nc.gpsimd.partition_broadcast(rbb[:], rc[0:1, :, :])
            nc.vector.tensor_tensor(
                yT_sb[0:D, pr, islice], osb[0:D, 0, :], rbb[:, 0, :], MULT
            )
            tmp = epool.tile([D, TI], BF16, tag="tmp")
            nc.vector.tensor_tensor(tmp[:], osb[0:D, 1, :], rbb[:, 1, :], MULT)
            # odd head's y^T lives on partitions 64-127: cross-partition
            # move must go through DMA
            nc.sync.dma_start(yT_sb[D:P, pr, islice], tmp[:])

        # ---- prologue compute: pair 0's chunk-0 q/k and V tile 0 ----
        for key, f in qkv_group_items(0, 1, 0) + qkv_group_items(0, 0, 0) + v_tile_items(0):
            f()
            if key is not None:
                done.add(key)

        # phase-0 filler: remaining chunk-0 tiles (all ready once their
        # DMAs land), then chunk-1 work
        for tt in (1, 2):
            queue += v_tile_items(tt)
        queue += qkv_group_items(1, 1, 0)
        queue += qkv_group_items(1, 0, 0)
        queue += v_tile_items(3)
        queue += qkv_group_items(2, 1, 0)
        queue += qkv_group_items(2, 0, 0)
        queue += qkv_group_items(3, 1, 0)
        queue += qkv_group_items(3, 0, 0)

        for it in range(NIT):
            njt = 4 * (it + 1)
            if it + 1 < NIT:
                for pr in range(JQ):
                    queue += qkv_group_items(pr, 1, it + 1)
                    queue += qkv_group_items(pr, 0, it + 1)
                for tt in range(4 * (it + 1), 4 * (it + 2)):
                    queue += v_tile_items(tt)
            if it > 0:
                for tt in range(4 * (it - 1), 4 * it):
                    for oc in range(NOC):
                        queue += cproj_items(tt, oc)
            for pr in range(JQ):
                po = o_ps.tile([P, 2, TI], FP32, tag="po")
                for jt in range(njt):
                    slot(pr, it, jt, njt, po)
                    if it == 0:
                        drain(7)
                    else:
                        n = max(2, -(-len(queue) // max(1, slots_left[0])))
                        drain(min(n, 8))
                    slots_left[0] -= 1
                epilogue(pr, it, po)
                drain(3 if it > 0 else 7)
                slots_left[0] -= 1

        # tail: c_proj for the last chunk
        for tt in range(4 * (NIT - 1), 4 * NIT):
            for oc in range(NOC):
                queue += cproj_items(tt, oc)
        drain(len(queue))


def build_nc():
    nc = bacc.Bacc("TRN2", target_bir_lowering=False, debug=False)
    io = {
        "xT": nc.dram_tensor("xT", [C, T], BF16, kind="ExternalInput").ap(),
        "wqkv": nc.dram_tensor("wqkv", [C, 3 * CL], BF16, kind="ExternalInput").ap(),
        "bqk": nc.dram_tensor("bqk", [P, 2 * JQ], FP32, kind="ExternalInput").ap(),
        "bv": nc.dram_tensor("bv", [1, CL], FP32, kind="ExternalInput").ap(),
        "wp": nc.dram_tensor("wp", [CL, C], BF16, kind="ExternalInput").ap(),
        "mtri": nc.dram_tensor("mtri", [P, P], FP32, kind="ExternalInput").ap(),
        # one partial [T, C] per core (c_proj contracted over this core's
        # 8 heads); the host sums the two per-batch partials in fp32
        "out": nc.dram_tensor("out", [T, C], BF16, kind="ExternalOutput").ap(),
    }
    with tile.TileContext(nc) as tc:
        _emit(tc, io)
    # The act-table-load pass assigns each activation the FIRST table set
    # containing its function, so Exp->'exp_and_others' and
    # Ln->'natural_log' alternate (a 1.3us ACT_TABLE_LOAD per switch).
    # Restrict the choice to 'natural_log_exp_and_others' (which holds
    # every function this kernel uses) so exactly one table load is
    # emitted.  Set ids stay aligned with act_info.json because the dict
    # keeps all entries in order.
    orig_tables = bacc.get_activation_tables

    def _combined_only(arch):
        t = orig_tables(arch)
        return {
            name: (funcs if name == "natural_log_exp_and_others" else set())
            for name, funcs in t.items()
        }

    bacc.get_activation_tables = _combined_only
    try:
        nc.compile()
    finally:
        bacc.get_activation_tables = orig_tables
    return nc


def make_in_maps(x, W_attn, b_attn, W_proj):
    """Per-core input dicts: core = 2*batch + head_group."""
    bf = ml_dtypes.bfloat16
    scale = np.float32(1.0 / math.sqrt(D))
    mtri = np.where(
        np.arange(P)[None, :] < np.arange(P)[:, None],
        np.float32(NEG),
        np.float32(0.0),
    ).astype(np.float32)
    in_maps = []
    for core in range(NCORES):
        b, g = divmod(core, 2)
        hs = slice(g * CL, (g + 1) * CL)
        wq = (W_attn[:, 0:C][:, hs] * scale).astype(bf)
        wk = W_attn[:, C : 2 * C][:, hs].astype(bf)
        wv = W_attn[:, 2 * C : 3 * C][:, hs].astype(bf)
        bq = (b_attn[0:C][hs] * scale).astype(np.float32)
        bk = b_attn[C : 2 * C][hs].astype(np.float32)
        bv = b_attn[2 * C : 3 * C][hs].astype(np.float32)
        in_maps.append(
            {
                "xT": np.ascontiguousarray(x[b].T).astype(bf),
                "wqkv": np.ascontiguousarray(np.concatenate([wq, wk, wv], axis=1)),
                "bqk": np.ascontiguousarray(
                    np.concatenate([bq, bk]).reshape(2 * JQ, P).T
                ),
                "bv": bv.reshape(1, CL),
                "wp": np.ascontiguousarray(W_proj[hs, :]).astype(bf),
                "mtri": mtri,
            }
        )
    return in_maps


def combine_outputs(results, b_proj):
    out = np.empty((B, T, C), np.float32)
    for b in range(B):
        acc = results[2 * b]["out"].astype(np.float32)
        acc = acc + results[2 * b + 1]["out"].astype(np.float32)
        acc += b_proj.astype(np.float32)[None, :]
        out[b] = acc
    return out


def _mask_is_causal(mask):
    if mask.shape != (B, T, T):
        return False
    tril = np.tril(np.ones((T, T), np.float32))
    return all(np.array_equal(np.asarray(mask[b]), tril) for b in range(B))


def _numpy_fallback(x, mask, W_attn, b_attn, W_proj, b_proj):
    # generic-mask fallback (never hit for the causal reference inputs)
    out = np.empty((B, T, C), np.float32)
    for b in range(B):
        qkv = x[b] @ W_attn + b_attn
        q, k, v = np.split(qkv, 3, axis=-1)
        q = q.reshape(T, H, D)
        k = k.reshape(T, H, D)
        v = v.reshape(T, H, D)
        y = np.empty((T, H, D), np.float32)
        for h in range(H):
            s = (q[:, h] @ k[:, h].T) / math.sqrt(D)
            s = s + NEG * (1.0 - mask[b])
            s = s - s.max(-1, keepdims=True)
            p = np.exp(s)
            p /= p.sum(-1, keepdims=True)
            y[:, h] = p @ v[:, h]
        out[b] = y.reshape(T, C) @ W_proj + b_proj
    return out


_NC = None


def kernel(x, mask, W_attn, b_attn, W_proj, b_proj):
    global _NC
    x = np.asarray(x, dtype=np.float32)
    mask = np.asarray(mask)
    W_attn = np.asarray(W_attn, dtype=np.float32)
    b_attn = np.asarray(b_attn, dtype=np.float32)
    W_proj = np.asarray(W_proj, dtype=np.float32)
    b_proj = np.asarray(b_proj, dtype=np.float32)

    if not _mask_is_causal(mask):
        return _numpy_fallback(x, mask, W_attn, b_attn, W_proj, b_proj)

    if _NC is None:
        _NC = build_nc()
    in_maps = make_in_maps(x, W_attn, b_attn, W_proj)
    res = run_bass_kernel_spmd(_NC, in_maps, core_ids=list(range(NCORES)))
    return combine_outputs(res.results, b_proj)
